# revision 43
# baseline (speedup 1.0000x reference)
"""Trainium2 Bass kernel for nn_AdaDepression (two-stage top-2 MoE router + sampler).

Data-parallel over batch (8 cores x 1024 tokens); weights/pools replicated.
All heavy matmuls run as bf16 hi/lo split pairs (error ~2^-16, full PE rate):
for operands a = ah + al, b = bh + bl we compute ah@bh + al@bh (stacked-K
matmuls) + ah@bl (correction), dropping al@bl (~2^-32).
Host supplies transposed hi/lo operands for x and the embedding tables; the
stage-2 selected embedding arrives pre-transposed via dma_gather(transpose).
exp on ACT with 1/||u|| folded into the activation scale; per-(token,expert)
combine weights w/Z on DVE; inverse-CDF sampling via tensor_tensor_scan +
scalar_tensor_tensor accum_out. Aux-loss partial sums reduce on host.
No collectives.
"""

import os
from contextlib import ExitStack

import numpy as np
import ml_dtypes

import concourse.bass as bass
import concourse.bacc as bacc
import concourse.tile as tile
import concourse.mybir as mybir
from concourse.bass_utils import run_bass_kernel_spmd

F32 = mybir.dt.float32
BF16 = mybir.dt.bfloat16
I32 = mybir.dt.int32
I16 = mybir.dt.int16
U32 = mybir.dt.uint32
AF = mybir.ActivationFunctionType
OP = mybir.AluOpType

B, N, D, H, E, K = 8192, 1024, 384, 64, 8, 2
NCORES = 8
BL = B // NCORES          # tokens per core
C = BL // 128             # 8 chunks of 128 tokens
AUX_COEF = 0.05

_CACHE = {}


def _build():
    nc = bacc.Bacc("TRN2", target_bir_lowering=False, num_devices=NCORES)

    def par(name, shape, dtype=F32, out=False):
        return nc.declare_dram_parameter(name, list(shape), dtype, isOutput=out)

    # x^T and emb^T hi/lo, host-transposed: [3*128, BL] bf16
    xt_hi_e = par("xt_hi", [3 * 128, BL], BF16)
    xt_lo_e = par("xt_lo", [3 * 128, BL], BF16)
    embt_e = [[par(f"embt{s}_{hl}", [3 * 128, N], BF16) for hl in "hl"]
              for s in range(2)]
    remb_e = par("remb", [N, D])                # row-major for dma_gather
    rand_e = par("rands", [2, BL])
    # A = [Wh;Wh] ([2K, F]), B = Wl ([K, F]), bias [2, F] = [bh; bl]
    gwA_e = [par("gwA1", [3 * 128, E], BF16), par("gwA2", [6 * 128, E], BF16)]
    gwB_e = [par("gwB1", [3 * 128, E], BF16), par("gwB2", [6 * 128, E], BF16)]
    gbhl_e = [par("gbhl1", [2, E], BF16), par("gbhl2", [2, E], BF16)]
    urA_e = [par("urA1", [3 * 128, E * H], BF16), par("urA2", [6 * 128, E * H], BF16)]
    urB_e = [par("urB1", [3 * 128, E * H], BF16), par("urB2", [6 * 128, E * H], BF16)]
    ubhl_e = [par("ubhl1", [2, E * H], BF16), par("ubhl2", [2, E * H], BF16)]
    vrA_e = [par("vrA1", [3 * 128, E * H], BF16), par("vrA2", [3 * 128, E * H], BF16)]
    vrB_e = [par("vrB1", [3 * 128, E * H], BF16), par("vrB2", [3 * 128, E * H], BF16)]
    vbhl_e = [par("vbhl1", [2, E * H], BF16), par("vbhl2", [2, E * H], BF16)]
    ident_e = par("ident", [128, 128])
    iota_n_e = par("iota_n", [128, N])
    iota_e_e = par("iota_e", [128, E])
    o_idx = par("o_idx", [2, BL], I32, out=True)
    o_logp = par("o_logp", [2, BL], out=True)
    o_aux = par("o_aux", [2, 128], out=True)
    idx_bounce = nc.dram_tensor("idx_bounce", [BL], I16)

    with tile.TileContext(nc) as tc, ExitStack() as ctx:
        pool = ctx.enter_context(tc.tile_pool(name="persist", bufs=1))

        # xT slots (d-chunks of 128 on partitions, tokens on free):
        # 0-2 xh, 3-5 xl, 6-8 sel_h, 9-11 sel_l
        xT = pool.tile([128, 12, BL], BF16)
        ident = pool.tile([128, 128], F32)
        iota_n = pool.tile([128, N], F32)
        iota_e = pool.tile([128, E], F32)
        ones2 = pool.tile([2, 128], BF16)
        ones_col = pool.tile([128, 1], F32)
        gwA_sb = [pool.tile([128, 6, E], BF16, name="gwA1t"),
                  pool.tile([128, 12, E], BF16, name="gwA2t")]
        gwB_sb = [pool.tile([128, 3, E], BF16, name="gwB1t"),
                  pool.tile([128, 6, E], BF16, name="gwB2t")]
        gb_sb = [pool.tile([2, E], BF16, name="gb1t"),
                 pool.tile([2, E], BF16, name="gb2t")]
        urA_sb = [pool.tile([128, 6, E * H], BF16, name="urA1t"),
                  pool.tile([128, 12, E * H], BF16, name="urA2t")]
        urB_sb = [pool.tile([128, 3, E * H], BF16, name="urB1t"),
                  pool.tile([128, 6, E * H], BF16, name="urB2t")]
        ub_sb = [pool.tile([2, E * H], BF16, name="ub1t"),
                 pool.tile([2, E * H], BF16, name="ub2t")]
        vrA_sb = [pool.tile([128, 6, E * H], BF16, name="vrA1t"),
                  pool.tile([128, 6, E * H], BF16, name="vrA2t")]
        vrB_sb = [pool.tile([128, 3, E * H], BF16, name="vrB1t"),
                  pool.tile([128, 3, E * H], BF16, name="vrB2t")]
        vb_sb = [pool.tile([2, E * H], BF16, name="vb1t"),
                 pool.tile([2, E * H], BF16, name="vb2t")]
        embt_sb0 = pool.tile([128, 6, N], BF16, name="embt0")
        embt_sb = [embt_sb0, embt_sb0]
        rand_sb = pool.tile([128, 2, C], F32)
        # v-hat^T: hh = [vh;vh] stacked on partitions, l = vl
        vT_hh = pool.tile([128, E, N], BF16)
        vT_l = pool.tile([64, E, N], BF16)
        # u^T stack per chunk: rows 0-63 uh, 64-127 ul
        uT = pool.tile([128, C, E * 128], BF16)
        idxw = pool.tile([128, 64], I16)
        sel_nat = pool.tile([128, C, D], F32)

        sync = nc.sync

        sync.dma_start(ident[:], ident_e[:])
        ehv0 = embt_e[0][0].rearrange("(k p) f -> p k f", p=128)
        elv0 = embt_e[0][1].rearrange("(k p) f -> p k f", p=128)
        for j in range(3):
            nc.scalar.dma_start(embt_sb[0][:, j, :], ehv0[:, j, :])
            nc.gpsimd.dma_start(embt_sb[0][:, 3 + j, :], elv0[:, j, :])
        sync.dma_start(iota_n[:], iota_n_e[:])
        sync.dma_start(iota_e[:], iota_e_e[:])
        nc.vector.memset(ones2[:], 1.0)
        nc.vector.memset(ones_col[:], 1.0)
        _dq = [nc.sync, nc.scalar, nc.gpsimd]
        _dqi = [0]
        def dq():
            _dqi[0] = (_dqi[0] + 1) % len(_dq)
            return _dq[_dqi[0]]

        def load_stage_weights(s):
            # A tiles ([Wh;Wh]): DRAM stores Wh once; DMA it into both halves
            for t_sb, t_e in [(vrA_sb, vrA_e), (gwA_sb, gwA_e), (urA_sb, urA_e)]:
                nk = t_sb[s].shape[1] // 2
                rview = t_e[s].rearrange("(k p) f -> p k f", p=128)
                for j in range(nk):
                    dq().dma_start(t_sb[s][:, j, :], rview[:, j, :])
                    dq().dma_start(t_sb[s][:, nk + j, :], rview[:, j, :])
            for t_sb, t_e in [(vrB_sb, vrB_e), (gwB_sb, gwB_e), (urB_sb, urB_e)]:
                rview = t_e[s].rearrange("(k p) f -> p k f", p=128)
                for j in range(t_sb[s].shape[1]):
                    dq().dma_start(t_sb[s][:, j, :], rview[:, j, :])
            for t_sb, t_e in [(gb_sb, gbhl_e), (ub_sb, ubhl_e), (vb_sb, vbhl_e)]:
                sync.dma_start(t_sb[s][:], t_e[s][:])

        load_stage_weights(0)
        sync.dma_start(rand_sb[:], rand_e.rearrange("s (c p) -> p s c", p=128))
        xhv = xt_hi_e.rearrange("(k p) b -> p k b", p=128)
        xlv = xt_lo_e.rearrange("(k p) b -> p k b", p=128)
        for j in range(3):
            dq().dma_start(xT[:, j, :], xhv[:, j, :])
            dq().dma_start(xT[:, 3 + j, :], xlv[:, j, :])

        def hl_matmul(out_ap, lhs_hi, lhs_lo, rhsA, rhsB, bias2):
            """out = sum((hi+lo chunks) @ A) + sum(hi chunks @ B) + ones2@bias."""
            for j, ap in enumerate(lhs_hi + lhs_lo):
                nc.tensor.matmul(out_ap, ap, rhsA[j], start=(j == 0), stop=False)
            for j, ap in enumerate(lhs_hi):
                nc.tensor.matmul(out_ap, ap, rhsB[j], start=False, stop=False)
            nc.tensor.matmul(out_ap, ones2[:], bias2, start=False, stop=True)

        for s in range(2):
            nd = 3 if s == 0 else 6
            if s == 0:
                hi_slots, lo_slots = [0, 1, 2], [3, 4, 5]
            else:
                hi_slots, lo_slots = [0, 1, 2, 6, 7, 8], [3, 4, 5, 9, 10, 11]
                load_stage_weights(1)

            # ---- v-hat^T build ----
            if s == 1:
                ehv = embt_e[s][0].rearrange("(k p) f -> p k f", p=128)
                elv = embt_e[s][1].rearrange("(k p) f -> p k f", p=128)
                for j in range(3):
                    dq().dma_start(embt_sb[s][:, j, :], ehv[:, j, :])
                    dq().dma_start(embt_sb[s][:, 3 + j, :], elv[:, j, :])
            with tc.tile_pool(name=f"vps_{s}", bufs=2, space="PSUM") as vps, \
                 tc.tile_pool(name=f"vtp_{s}", bufs=2, space="PSUM") as vtp, \
                 tc.tile_pool(name=f"vtmp_{s}", bufs=2) as vtmp:
                for nk in range(C):
                    vp = vps.tile([128, E * H], F32, tag="vp")
                    ehi = [embt_sb[s][:, j, nk * 128:(nk + 1) * 128]
                           for j in range(3)]
                    elo = [embt_sb[s][:, 3 + j, nk * 128:(nk + 1) * 128]
                           for j in range(3)]
                    hl_matmul(vp[:], ehi, elo,
                              [vrA_sb[s][:, j, :] for j in range(6)],
                              [vrB_sb[s][:, j, :] for j in range(3)],
                              vb_sb[s][:])
                    vsq = vtmp.tile([128, E * H], F32, tag="vsq")
                    nc.scalar.square(vsq[:], vp[:])
                    vss = vtmp.tile([128, E], F32, tag="vss")
                    nc.vector.tensor_reduce(
                        vss[:], vsq[:].rearrange("p (e h) -> p e h", e=E),
                        mybir.AxisListType.X, OP.add)
                    vssr = vtmp.tile([128, E], F32, tag="vssr")
                    nc.vector.reciprocal(vssr[:], vss[:])
                    vrn = vtmp.tile([128, E], F32, tag="vrn")
                    nc.scalar.sqrt(vrn[:], vssr[:])
                    vhat = vtmp.tile([128, E, H], F32, tag="vhat")
                    nc.vector.tensor_tensor(
                        vhat[:], vp[:].rearrange("p (e h) -> p e h", e=E),
                        vrn[:].unsqueeze(2).broadcast_to([128, E, H]), OP.mult)
                    tp = vtp.tile([64, E * 128], F32, tag="vtp")
                    for e in range(E):
                        nc.tensor.transpose(
                            tp[:, e * 128:(e + 1) * 128], vhat[:, e, :], ident[:])
                    tpv = tp[:].rearrange("h (e j) -> h e j", e=E)
                    nsl = slice(nk * 128, (nk + 1) * 128)
                    nc.scalar.copy(vT_hh[0:64, :, nsl], tpv)
                    nc.sync.dma_start(
                        vT_hh[64:128, :, nsl], vT_hh[0:64, :, nsl])
                    nc.vector.tensor_tensor(
                        vT_l[:, :, nsl], tpv, vT_hh[0:64, :, nsl], OP.subtract)

            with tc.tile_pool(name=f"sm_{s}", bufs=3) as sm, \
                 tc.tile_pool(name=f"big_{s}", bufs=4) as big, \
                 tc.tile_pool(name=f"acc_{s}", bufs=2) as accp, \
                 tc.tile_pool(name=f"st_{s}", bufs=1) as st:

                sel_f = st.tile([128, C], F32)
                psel = st.tile([128, C], F32)
                aux_sb = st.tile([128, 128], F32)
                logp = st.tile([128, C], F32)
                idx32 = st.tile([128, C], I32)
                w_all = st.tile([128, C, E], F32)
                ssr_all = st.tile([128, C, E], F32)
                rn_all = st.tile([128, C, E], F32)

                # ---- phase A: routing + u projection ----
                pa_ctx = ExitStack()
                lgp = pa_ctx.enter_context(
                    tc.tile_pool(name=f"lg_{s}", bufs=2, space="PSUM"))
                upp = pa_ctx.enter_context(
                    tc.tile_pool(name=f"up_{s}", bufs=2, space="PSUM"))
                utp = pa_ctx.enter_context(
                    tc.tile_pool(name=f"utp_{s}", bufs=2, space="PSUM"))
                lg_all = st.tile([128, C, E], F32)
                t8v_all = st.tile([128, C, 8], F32)
                t8i_all = st.tile([128, C, 8], U32)
                uss_all = st.tile([128, C, E], F32)
                for c in range(C):
                    csl = slice(c * 128, (c + 1) * 128)
                    xhi = [xT[:, j, csl] for j in hi_slots]
                    xlo = [xT[:, j, csl] for j in lo_slots]

                    lg = lgp.tile([128, E], F32, tag="lg")
                    hl_matmul(lg[:], xhi, xlo,
                              [gwA_sb[s][:, j, :] for j in range(2 * nd)],
                              [gwB_sb[s][:, j, :] for j in range(nd)],
                              gb_sb[s][:])
                    nc.vector.tensor_copy(lg_all[:, c, :], lg[:])
                    nc.vector.max(t8v_all[:, c, :], lg_all[:, c, :])
                    nc.vector.max_index(t8i_all[:, c, :], t8v_all[:, c, :],
                                        lg_all[:, c, :])

                    up = upp.tile([128, E * H], F32, tag="up")
                    hl_matmul(up[:], xhi, xlo,
                              [urA_sb[s][:, j, :] for j in range(2 * nd)],
                              [urB_sb[s][:, j, :] for j in range(nd)],
                              ub_sb[s][:])
                    u_sb = sm.tile([128, E * H], F32, tag="u_sb")
                    nc.scalar.copy(u_sb[:], up[:])
                    usq = sm.tile([128, E * H], F32, tag="usq")
                    nc.scalar.square(usq[:], up[:])
                    nc.vector.tensor_reduce(
                        uss_all[:, c, :], usq[:].rearrange("p (e h) -> p e h", e=E),
                        mybir.AxisListType.X, OP.add)
                    if c == 0:
                        nc.vector.reciprocal(ssr_all[:, 0, :], uss_all[:, 0, :])
                        nc.scalar.sqrt(rn_all[:, 0, :], ssr_all[:, 0, :])
                    ut_ps = utp.tile([64, E * 128], F32, tag="ut_ps")
                    for e in range(E):
                        nc.tensor.transpose(
                            ut_ps[:, e * 128:(e + 1) * 128],
                            u_sb[:, e * 64:(e + 1) * 64], ident[:])
                    nc.scalar.copy(uT[0:64, c, :], ut_ps[:])
                    nc.vector.tensor_tensor(
                        uT[64:128, c, :], ut_ps[:], uT[0:64, c, :], OP.subtract)

                # ---- batched routing epilogue ----
                ef = st.tile([128, C, 2], F32)
                nc.vector.tensor_copy(ef[:], t8i_all[:, :, 0:2])
                ev = st.tile([128, C, 2], F32)
                nc.scalar.activation(ev[:], t8v_all[:, :, 0:2], AF.Exp)
                es = st.tile([128, C], F32)
                nc.vector.tensor_reduce(es[:], ev[:], mybir.AxisListType.X, OP.add)
                esr = st.tile([128, C], F32)
                nc.vector.reciprocal(esr[:], es[:])
                gwk = st.tile([128, C, 2], F32)
                nc.vector.tensor_tensor(
                    gwk[:], ev[:],
                    esr[:].unsqueeze(2).broadcast_to([128, C, 2]), OP.mult)
                pr = st.tile([128, C, E], F32)
                nc.scalar.activation(
                    pr[:].rearrange("p c e -> p (c e)"),
                    lg_all[:].rearrange("p c e -> p (c e)"), AF.Exp)
                zl = st.tile([128, C], F32)
                nc.vector.tensor_reduce(zl[:], pr[:], mybir.AxisListType.X, OP.add)
                zlr = st.tile([128, C], F32)
                nc.vector.reciprocal(zlr[:], zl[:])
                nc.vector.tensor_tensor(
                    aux_sb[:, 0:64].rearrange("p (c e) -> p c e", c=C), pr[:],
                    zlr[:].unsqueeze(2).broadcast_to([128, C, E]), OP.mult)
                iota_ce = iota_e[:].unsqueeze(1).broadcast_to([128, C, E])
                mA = st.tile([128, C, E], F32)
                mB = st.tile([128, C, E], F32)
                nc.vector.tensor_tensor(
                    mA[:], iota_ce,
                    ef[:, :, 0:1].broadcast_to([128, C, E]), OP.is_equal)
                nc.vector.tensor_tensor(
                    mB[:], iota_ce,
                    ef[:, :, 1:2].broadcast_to([128, C, E]), OP.is_equal)
                nc.vector.tensor_tensor(
                    aux_sb[:, 64:128].rearrange("p (c e) -> p c e", c=C),
                    mA[:], mB[:], OP.add)
                nc.vector.tensor_tensor(
                    w_all[:], mA[:],
                    gwk[:, :, 0:1].broadcast_to([128, C, E]), OP.mult)
                wB = st.tile([128, C, E], F32)
                nc.vector.tensor_tensor(
                    wB[:], mB[:],
                    gwk[:, :, 1:2].broadcast_to([128, C, E]), OP.mult)
                nc.vector.tensor_tensor(w_all[:], w_all[:], wB[:], OP.add)
                nc.vector.reciprocal(
                    ssr_all[:, 1:, :].rearrange("p c e -> p (c e)"),
                    uss_all[:, 1:, :].rearrange("p c e -> p (c e)"))

                auxp = lgp.tile([128, E], F32, tag="lg")
                nc.tensor.matmul(auxp[:, 0:1], aux_sb[:], ones_col[:],
                                 start=True, stop=True)
                aux_out = st.tile([128, 1], F32)
                nc.vector.tensor_copy(aux_out[:], auxp[:, 0:1])
                sync.dma_start(o_aux[s], aux_out[:])
                pa_ctx.close()

                # batched sqrt for chunks 1..C-1 (chunk 0 done eagerly)
                nc.scalar.sqrt(rn_all[:, 1:, :].rearrange("p c e -> p (c e)"),
                               ssr_all[:, 1:, :].rearrange("p c e -> p (c e)"))

                # ---- phase B: scores/exp/combine/sample ----
                with tc.tile_pool(name=f"scp_{s}", bufs=3, space="PSUM") as scp:
                    for c in range(C):
                        acc = accp.tile([128, N], F32, tag="acc")
                        Zc = sm.tile([128, E], F32, tag="Zc")
                        al = sm.tile([128, E], F32, tag="al")
                        Ets = [None] * E
                        for e in range(E):
                            sc = scp.tile([128, N], F32, tag="sc")
                            l1 = uT[:, c, e * 128:(e + 1) * 128]
                            l2 = uT[0:64, c, e * 128:(e + 1) * 128]
                            for half in range(2):
                                nsl = slice(half * 512, (half + 1) * 512)
                                nc.tensor.matmul(
                                    sc[:, nsl], l1, vT_hh[:, e, nsl],
                                    start=True, stop=False)
                                nc.tensor.matmul(
                                    sc[:, nsl], l2, vT_l[:, e, nsl],
                                    start=False, stop=True)
                            Et = Ets[e] = big.tile([128, N], F32, tag="Et", name="Et")
                            nc.scalar.activation(
                                Et[:], sc[:], AF.Exp,
                                scale=rn_all[:, c, e:e + 1],
                                accum_out=Zc[:, e:e + 1])
                            if e % 2 == 1:
                                pe = slice(e - 1, e + 1)
                                nc.vector.reciprocal(al[:, pe], Zc[:, pe])
                                nc.vector.tensor_tensor(
                                    al[:, pe], w_all[:, c, pe], al[:, pe],
                                    OP.mult)
                                for ee in (e - 1, e):
                                    if ee == 0:
                                        nc.scalar.activation(
                                            acc[:], Ets[0][:], AF.Copy,
                                            scale=al[:, 0:1])
                                    else:
                                        nc.vector.scalar_tensor_tensor(
                                            acc[:], Ets[ee][:], al[:, ee:ee + 1],
                                            acc[:], OP.mult, OP.add)

                        cum = accp.tile([128, N], F32, tag="cum")
                        nc.vector.tensor_tensor_scan(
                            cum[:], acc[:], acc[:], 0.0, OP.add, OP.bypass)
                        scr = accp.tile([128, N], F32, tag="scr")
                        nc.vector.scalar_tensor_tensor(
                            scr[:], cum[:], rand_sb[:, s, c:c + 1], cum[:],
                            OP.is_le, OP.bypass, accum_out=sel_f[:, c:c + 1])
                        nc.vector.scalar_tensor_tensor(
                            scr[:], iota_n[:], sel_f[:, c:c + 1], acc[:],
                            OP.is_equal, OP.mult, accum_out=psel[:, c:c + 1])

                nc.vector.tensor_scalar(psel[:], psel[:], 1e-38, None, OP.max)
                nc.scalar.activation(logp[:], psel[:], AF.Ln)
                nc.vector.tensor_copy(idx32[:], sel_f[:])
                sync.dma_start(o_idx[s].rearrange("(c p) -> p c", p=128), idx32[:])
                sync.dma_start(o_logp[s].rearrange("(c p) -> p c", p=128), logp[:])

                if s == 0:
                    idx16 = st.tile([128, C], I16)
                    nc.vector.tensor_copy(idx16[:], sel_f[:])
                    # write DRAM in wrapped order: token r=c*128+p -> slot
                    # (p%16)*64 + c*8 + p//16
                    wrout = idx_bounce.rearrange("(q c g) -> g q c", q=16, c=C)
                    sync.dma_start(wrout, idx16[:])
                    wrin = idx_bounce.rearrange("(q f) -> q f", q=16)
                    sync.dma_start(
                        idxw[:], wrin.unsqueeze(0).broadcast_to([8, 16, 64]))
                    nc.gpsimd.dma_gather(
                        sel_nat[:], remb_e[:], idxw[:],
                        num_idxs=BL, num_idxs_reg=BL, elem_size=D)

            if s == 0:
                with tc.tile_pool(name="selt", bufs=2, space="PSUM") as stp:
                    for c in range(C):
                        for dc in range(3):
                            t = stp.tile([128, 128], F32, tag="selt")
                            nc.tensor.transpose(
                                t[:], sel_nat[:, c, dc * 128:(dc + 1) * 128],
                                ident[:])
                            hi = xT[:, 6 + dc, c * 128:(c + 1) * 128]
                            nc.scalar.copy(hi, t[:])
                            nc.vector.tensor_tensor(
                                xT[:, 9 + dc, c * 128:(c + 1) * 128], t[:], hi,
                                OP.subtract)

    nc.compile()
    return nc


def _hl(a):
    """Split float32 array -> (hi, lo) bf16."""
    hi = a.astype(ml_dtypes.bfloat16)
    lo = (a - hi.astype(np.float32)).astype(ml_dtypes.bfloat16)
    return hi, lo


def _stackA(w):
    """[K, F] f32 -> A=[K, F] bf16 (Wh; loaded twice on device), B=Wl."""
    return _hl(w)


def _prep(inputs):
    f = np.float32
    xt = np.ascontiguousarray(np.asarray(inputs["x"], f).T)
    xth, xtl = _hl(xt)
    common = {}
    for s, emb in [(0, "rea_emb"), (1, "llm_emb")]:
        et = np.ascontiguousarray(np.asarray(inputs[emb], f).T)
        eh, el = _hl(et)
        common[f"embt{s}_h"] = eh
        common[f"embt{s}_l"] = el
    common["remb"] = np.ascontiguousarray(np.asarray(inputs["rea_emb"], f))

    for s, (gw, gb, ur, ub, vr, vb) in enumerate([
        ("gate_rea_w", "gate_rea_b", "Ur_rea", "Ub_rea", "Vr_rea", "Vb_rea"),
        ("gate_llm_w", "gate_llm_b", "Ur_llm", "Ub_llm", "Vr_llm", "Vb_llm"),
    ]):
        gwA, gwB = _stackA(np.asarray(inputs[gw], f))
        common[f"gwA{s+1}"], common[f"gwB{s+1}"] = gwA, gwB
        gbh, gbl = _hl(np.asarray(inputs[gb], f))
        common[f"gbhl{s+1}"] = np.stack([gbh, gbl])
        urT = np.ascontiguousarray(
            np.asarray(inputs[ur], f).transpose(1, 0, 2).reshape(-1, E * H))
        common[f"urA{s+1}"], common[f"urB{s+1}"] = _stackA(urT)
        ubh, ubl = _hl(np.asarray(inputs[ub], f).reshape(E * H))
        common[f"ubhl{s+1}"] = np.stack([ubh, ubl])
        vrT = np.ascontiguousarray(
            np.asarray(inputs[vr], f).transpose(1, 0, 2).reshape(-1, E * H))
        common[f"vrA{s+1}"], common[f"vrB{s+1}"] = _stackA(vrT)
        vbh, vbl = _hl(np.asarray(inputs[vb], f).reshape(E * H))
        common[f"vbhl{s+1}"] = np.stack([vbh, vbl])

    common["ident"] = np.eye(128, dtype=f)
    common["iota_n"] = np.ascontiguousarray(
        np.broadcast_to(np.arange(N, dtype=f), (128, N)))
    common["iota_e"] = np.ascontiguousarray(
        np.broadcast_to(np.arange(E, dtype=f), (128, E)))

    maps = []
    for i in range(NCORES):
        sl = slice(i * BL, (i + 1) * BL)
        m = dict(common)
        m["xt_hi"] = np.ascontiguousarray(xth[:, sl])
        m["xt_lo"] = np.ascontiguousarray(xtl[:, sl])
        m["rands"] = np.stack([
            np.asarray(inputs["rand_rea"], f)[sl, 0],
            np.asarray(inputs["rand_llm"], f)[sl, 0]])
        maps.append(m)
    return maps


def kernel(**inputs):
    inputs = {k: np.asarray(v) for k, v in inputs.items()}
    if "nc" not in _CACHE:
        _CACHE["nc"] = _build()
    nc = _CACHE["nc"]
    in_maps = _prep(inputs)
    res = run_bass_kernel_spmd(
        nc, in_maps, core_ids=list(range(NCORES)),
        trace=bool(os.environ.get("KTRACE")))
    _CACHE["last"] = res
    outs = res.results
    rea_idx = np.concatenate([o["o_idx"][0] for o in outs]).astype(np.int32)
    llm_idx = np.concatenate([o["o_idx"][1] for o in outs]).astype(np.int32)
    rea_logp = np.concatenate([o["o_logp"][0] for o in outs]).astype(
        np.float32)[:, None]
    llm_logp = np.concatenate([o["o_logp"][1] for o in outs]).astype(
        np.float32)[:, None]
    aux = np.float32(0.0)
    for s in range(2):
        acc = np.zeros(128, np.float64)
        for o in outs:
            acc += o["o_aux"][s]
        pm = acc[:64].reshape(C, E).sum(0) / B
        mm = acc[64:].reshape(C, E).sum(0) / B
        aux = np.float32(aux + E * AUX_COEF * np.float32((pm * mm).sum()))
    return rea_idx, rea_logp, llm_idx, llm_logp, aux
